# revision 46
# baseline (speedup 1.0000x reference)
"""Trainium2 Bass kernel for BilinearInteractionPlusLayer.

Math (per batch row b):
    pairs (i,j), i<j over F=40 fields, P=C(40,2)=780 pairs
    t[b,p,f] = sum_e x[b,i,e] * W[p,e,f]
    q[b,p]   = sum_f t[b,p,f] * x[b,j,f]
    h[b,d]   = sum_p q[b,p] * dense_w[p,d] + dense_b[d]
    out      = LayerNorm(h) * gamma + beta          (eps = 1e-3)

Sharding: data-parallel over batch, 2048 -> 256 rows on each of 8 cores.
W / dense_w / LN params are replicated. No collectives.

Per-core pipeline (pair math in a transposed "[feature x batch]" layout):
  - Stage 1 (PE): per (field i, j-group jg) "chunk", one matmul
        lhsT = Wcat chunk [32e x 128(c,f)]  (host-packed, zeros for absent
               pairs, placed on partition strip i%4)
        rhs  = xT_i      [32e x 256b]
        out  = t chunk   [128(c,f) x 256b] in PSUM (fp32; TRN2 PSUM is
               fp32-only)
  - The elementwise q-multiply m = t * xj is routed per segment across
    three engine paths, statically balanced against measured HW rates:
        'act':  ACT casts t -> bf16 SBUF, DVE tensor_mul at the 2-byte
                2x fast path
        'pool': ACT casts, Pool (GpSimd) multiplies from SBUF
        'dve':  DVE multiplies fp32-from-PSUM directly (1x, no cast)
    Segments are 2 chunks (one PSUM bank) x 7 pool bufs, so a full
    4-strip step allocates without stalling the in-order PE queue.
  - PE folds dense_w AND the (pair,f) reduction into accumulating matmuls
    (4 column-strip partial accumulators run via tile_position packing).
  - Tail: transpose-accumulate into [b x d], LayerNorm via bn_stats/bn_aggr.
Startup: input DMAs are split across the two hardware DMA queues (Sync +
Scalar engines) in first-needed-first order, with the weight tensors split
into independently-DMA'd tiles so early matmuls only wait on their own
slice; the Sqrt activation table (which also contains Copy) is preloaded
with a dummy op so the LayerNorm tail doesn't pay a 1.3us table switch.
"""

import itertools

import numpy as np

import concourse.bass as bass
from concourse import bacc, mybir
from concourse.bass_utils import run_bass_kernel_spmd
from concourse.tile import TileContext

F32 = mybir.dt.float32
BF16 = mybir.dt.bfloat16
NP_BF16 = mybir.dt.np(BF16)

B, F, E, P, D = 2048, 40, 32, 780, 16
NCORES = 8
BS = B // NCORES          # 256 batch rows per core
NJG = F // 4              # 10 j-groups of 4 fields
LN_EPS = 1e-3
SEG = 2                   # j-group chunks per pipeline segment

# wsb tile boundaries (in slots) / dw2 tile boundaries (in chunks): each
# range is one SBUF tile fed by one DMA, so consumers only wait for the
# slice they read. All but the first are DMA'd from inside the main loop
# so early matmuls don't sit behind them on the queue's completion
# counter.
WSB_CUTS = [0, 12, 24, 36, 55]
DW2_CUTS = [0, 48, 112, 210]


def _make_chunks():
    """One chunk = (field i, j-group jg): a [32e x 128(c,f)] stage-1 matmul."""
    chunks = []
    per_strip = [0, 0, 0, 0]
    for i in range(F):
        r = i % 4
        for jg in range((i + 1) // 4, NJG):
            chunks.append(
                {"i": i, "jg": jg, "r": r, "s": per_strip[r], "k": len(chunks)}
            )
            per_strip[r] += 1
    return chunks, per_strip


CHUNKS, PER_STRIP = _make_chunks()
CH_BY_IJG = {(c["i"], c["jg"]): c for c in CHUNKS}
NCH = len(CHUNKS)          # 210
SLOTS = max(PER_STRIP)     # 55 weight-chunk slots per partition strip


def _segments_of(i):
    jgs = list(range((i + 1) // 4, NJG))
    return [jgs[s:s + SEG] for s in range(0, len(jgs), SEG)]


def _host_weights(W, dense_w):
    """Pack W into per-strip stationary chunks and dense_w into per-chunk
    [128(c,f) x 32] reducers (cols 0:16 = dense_w broadcast over f, cols
    16:32 = zeros so the dense matmuls write full 32-partition strips of
    h4, letting the tail drain h4 with one full-width copy)."""
    pair_idx = {pq: n for n, pq in enumerate(itertools.combinations(range(F), 2))}
    wsb = np.zeros((128, SLOTS * 128), np.float32)
    dw2 = np.zeros((128, NCH * 32), np.float32)
    for ch in CHUNKS:
        i, jg, r, s, k = ch["i"], ch["jg"], ch["r"], ch["s"], ch["k"]
        for c in range(4):
            j = 4 * jg + c
            if j <= i:
                continue
            p = pair_idx[(i, j)]
            wsb[32 * r:32 * r + 32, s * 128 + 32 * c:s * 128 + 32 * c + 32] = W[p]
            dw2[32 * c:32 * c + 32, k * 32:k * 32 + 16] = dense_w[p][None, :]
    return wsb.astype(NP_BF16), dw2.astype(NP_BF16)


def _host_xt(xc):
    """Per-core phase layout: xt[32*(j%4)+f, (j//4)*BS + b] = xc[b, j, f]."""
    arr = xc.transpose(1, 2, 0)                    # [F, E, BS]
    arr = arr.reshape(NJG, 4, E, BS)               # [jg, c, f, b]
    arr = arr.transpose(1, 2, 0, 3)                # [c, f, jg, b]
    return np.ascontiguousarray(arr.reshape(128, NJG * BS)).astype(NP_BF16)


def _host_ident4():
    """[128, 16] with a 16x16 identity at each 32-partition strip."""
    id4 = np.zeros((128, 16), np.float32)
    for g in range(4):
        id4[32 * g:32 * g + 16, :] = np.eye(16, dtype=np.float32)
    return id4


def _build_steps():
    """Baseline step schedule: quads of consecutive fields, software
    pipelined; returns the list of steps, each a list of (i, seg-jgs)."""
    steps = []
    for t0 in range(0, F, 4):
        quad = [i for i in range(t0, min(t0 + 4, F))]
        segl = {i: _segments_of(i) for i in quad}
        nsteps = max(len(s) for s in segl.values())
        for step in range(nsteps):
            steps.append([(i, segl[i][step]) for i in quad
                          if step < len(segl[i])])
    return steps


def _route_schedule(steps):
    """Statically balance the elementwise work across ACT / DVE / POOL.
    Returns {group_index: route} where groups enumerate (i, seg) in
    emission order. Routes: 'act' (ACT cast + DVE fast mul), 'pool'
    (Pool cast + DVE fast mul), 'dve' (DVE direct fp32 mul)."""
    # per-engine busy seeds (ns): tail work living on each engine
    busy = {"ACT": 2600.0, "DVE": 2400.0, "POOL": 400.0}
    routes = []
    for active in steps:
        for i, seg in active:
            n = len(seg)
            # measured per-chunk rates: ACT cast 213, DVE bf16 mul 133 (2x),
            # DVE direct fp32 mul 267 (1x), Pool bf16 mul 508
            cand = {
                "act": {"ACT": 213 * n + 260, "DVE": 133 * n + 125},
                "pool": {"ACT": 213 * n + 260, "POOL": 508 * n + 270},
                "dve": {"DVE": 267 * n + 150},
            }
            best, best_max = None, None
            for r, costs in cand.items():
                trial = dict(busy)
                for eng, c in costs.items():
                    trial[eng] += c
                m = max(trial.values())
                if best_max is None or m < best_max:
                    best, best_max = r, m
            for eng, c in cand[best].items():
                busy[eng] += c
            routes.append(best)
    return routes, busy


def _build_bass():
    nc = bacc.Bacc(trn_type="TRN2")
    xin = nc.dram_tensor("xt", [128, NJG * BS], BF16, kind="ExternalInput")
    wsb = nc.dram_tensor("wsb", [128, SLOTS * 128], BF16, kind="ExternalInput")
    dw2 = nc.dram_tensor("dw2", [128, NCH * 32], BF16, kind="ExternalInput")
    vecs = nc.dram_tensor("vecs", [3, D], F32, kind="ExternalInput")
    id4 = nc.dram_tensor("ident4", [128, D], F32, kind="ExternalInput")
    out = nc.dram_tensor("out", [BS, D], F32, kind="ExternalOutput")

    steps = _build_steps()
    routes, pred_busy = _route_schedule(steps)
    import collections
    import sys
    print(f"route mix: {dict(collections.Counter(routes))}  "
          f"predicted busy(ns): { {k: int(v) for k, v in pred_busy.items()} }",
          file=sys.stderr)

    with TileContext(nc) as tc:
        with (
            tc.tile_pool(name="const", bufs=1) as const,
            tc.tile_pool(name="cast", bufs=12) as cbuf,
            tc.tile_pool(name="mbuf", bufs=18) as mbuf,
            tc.tile_pool(name="tsegp", bufs=7, space="PSUM") as tsegp,
            tc.tile_pool(name="hpsp", bufs=1, space="PSUM") as hpsp,
            tc.tile_pool(name="lnp", bufs=2) as lnp,
        ):
            # ---- inputs. The Sync hardware queue carries ONLY what the
            # first matmuls need (the tile scheduler hoists every queued
            # DMA ahead of compute, and consumers wait on the queue's
            # completion count, so anything extra here delays the start).
            # Everything else rides the GpSimd software DGE in
            # consumption order. The Scalar engine gets no DMAs (a DMA
            # occupies the issuing engine for the whole transfer, which
            # would starve the casts).
            xT = const.tile([128, NJG, BS], BF16)
            wsb_tiles = [
                (a, b_, const.tile([128, (b_ - a) * 128], BF16, name=f"wsb{a}"))
                for a, b_ in zip(WSB_CUTS[:-1], WSB_CUTS[1:])
            ]
            dw2_tiles = [
                (a, b_, const.tile([128, (b_ - a) * 32], BF16, name=f"dw2{a}"))
                for a, b_ in zip(DW2_CUTS[:-1], DW2_CUTS[1:])
            ]
            nc.sync.dma_start(out=xT[:, 0:2, :], in_=xin[:, 0:2 * BS])
            a, b_, t_ = wsb_tiles[0]
            nc.sync.dma_start(out=t_[:], in_=wsb[:, a * 128:b_ * 128])
            nc.sync.dma_start(out=xT[:, 2:NJG, :], in_=xin[:, 2 * BS:])
            vec_t = const.tile([128, 3, D], F32)
            id4_t = const.tile([128, D], F32)
            # Late DMAs are emitted from inside the loop; each is pinned
            # behind real compute by a 1-element guard copy into its
            # destination (a WAW dep the scheduler cannot hoist past), so
            # early matmuls never wait on the shared DMA counter for them.
            def emit_late_dma(kind, arg, last_m):
                if kind == "wsb":
                    a, b_, t_ = arg
                    nc.gpsimd.tensor_copy(out=t_[0:1, 0:1],
                                          in_=last_m[0:1, 0, 0:1])
                    nc.sync.dma_start(out=t_[:], in_=wsb[:, a * 128:b_ * 128])
                elif kind == "dw2":
                    a, b_, t_ = arg
                    nc.gpsimd.tensor_copy(out=t_[0:1, 0:1],
                                          in_=last_m[0:1, 0, 0:1])
                    nc.sync.dma_start(out=t_[:], in_=dw2[:, a * 32:b_ * 32])
                elif kind == "vecs":
                    # rows: 0 = dense_b, 1 = gamma, 2 = beta; broadcast
                    # across partitions
                    nc.gpsimd.tensor_copy(out=vec_t[0:1, 0, 0:1],
                                          in_=last_m[0:1, 0, 0:1])
                    src = vecs[:, :]
                    nc.sync.dma_start(
                        out=vec_t[:],
                        in_=bass.AP(tensor=src.tensor, offset=src.offset,
                                    ap=[[0, 128]] + [list(a) for a in src.ap]),
                    )
                else:
                    nc.gpsimd.tensor_copy(out=id4_t[0:1, 0:1],
                                          in_=last_m[0:1, 0, 0:1])
                    nc.sync.dma_start(out=id4_t[:], in_=id4[:, :])

            late_dmas = {
                1: [("dw2", dw2_tiles[0]), ("wsb", wsb_tiles[1])],
                3: [("dw2", dw2_tiles[1]), ("wsb", wsb_tiles[2])],
                5: [("dw2", dw2_tiles[2]), ("wsb", wsb_tiles[3])],
                7: [("vecs", None)],
                9: [("id4", None)],
            }
            eps_t = const.tile([128, 1], F32)
            nc.vector.memset(eps_t[:], LN_EPS)

            def wsb_ap(s):
                for a, b_, t_ in wsb_tiles:
                    if a <= s < b_:
                        return t_, (s - a)
                raise AssertionError(s)

            def dw2_ap(k):
                for a, b_, t_ in dw2_tiles:
                    if a <= k < b_:
                        return t_, (k - a)
                raise AssertionError(k)

            def rr_order(items):
                """Round-robin dw2 items across the 4 column strips so
                adjacent matmuls hit different PE column tiles."""
                buckets = {g: [] for g in range(4)}
                for it in items:
                    buckets[it[2] % 4].append(it)
                out_ = []
                while any(buckets.values()):
                    for g in range(4):
                        if buckets[g]:
                            out_.append(buckets[g].pop(0))
                return out_

            # One PSUM bank holds: h4 (four col-tiled partial accumulators
            # [16d x 256b] at partition strips, free 0:256), hsum ([16d x
            # 256b], free 256:512), and ht reuses h4's range after h4 is
            # consumed.
            hcomb = hpsp.tile([128, 2 * BS], F32)
            h4 = hcomb[:, 0:BS]
            hsum = hcomb[0:D, BS:2 * BS]

            # Prepass: replay the flush schedule to learn the actual
            # per-strip first/last dw2 matmul (for PSUM start/stop flags).
            sim_pending = []
            sim_order = []

            def sim_flush():
                items = [(None, u, CH_BY_IJG[(i, jg)]["k"])
                         for (i, seg) in sim_pending for u, jg in enumerate(seg)]
                sim_order.extend(k for _, _, k in rr_order(items))
                sim_pending.clear()

            for sidx, active in enumerate(steps):
                if sidx % 4 == 0 and sidx > 0:
                    sim_flush()
                sim_pending.extend(active)
            sim_flush()
            first_k = {}
            last_k = {}
            for k in sim_order:
                first_k.setdefault(k % 4, k)
                last_k[k % 4] = k

            pending = []          # list of (m_tile, [(u, k), ...])

            def flush_dw2():
                items = [(m_t, u, k) for m_t, ks in pending for u, k in ks]
                for m_t, u, k in rr_order(items):
                    g = k % 4
                    dtile, koff = dw2_ap(k)
                    nc.tensor.matmul(
                        h4[32 * g:32 * g + 32, :],
                        lhsT=dtile[:, koff * 32:(koff + 1) * 32],
                        rhs=m_t[:, u, :],
                        start=(first_k[g] == k),
                        stop=(last_k[g] == k),
                        tile_position=(0, 32 * g),
                    )
                pending.clear()

            gidx = 0
            for sidx, active in enumerate(steps):
                tps = {i: tsegp.tile([128, SEG, BS], F32, tag="t",
                                     name="tseg")
                       for i, _ in active}
                # stage-1 matmul burst, chunk-interleaved across strips
                maxc = max(len(seg) for _, seg in active)
                for u in range(maxc):
                    for i, seg in active:
                        if u >= len(seg):
                            continue
                        ch = CH_BY_IJG[(i, seg[u])]
                        r = i % 4
                        wtile, soff = wsb_ap(ch["s"])
                        nc.tensor.matmul(
                            tps[i][:, u, :],
                            lhsT=wtile[32 * r:32 * r + 32,
                                       soff * 128:(soff + 1) * 128],
                            rhs=xT[32 * r:32 * r + 32, i // 4, :],
                            start=True, stop=True,
                            tile_position=(32 * r, 0),
                        )
                # dw2 matmuls are flushed every other step (with >=1 step of
                # lag so the m tiles are ready): longer same-kind PE runs
                # mean fewer row-tile <-> column-tile transitions, which
                # serialize the PE array.
                if sidx % 4 == 0 and sidx > 0:
                    flush_dw2()
                # elementwise multiply, statically routed
                for i, seg in active:
                    n = len(seg)
                    route = routes[gidx]
                    gidx += 1
                    m_t = mbuf.tile([128, SEG, BS], BF16, tag="m")
                    if route == "dve":
                        nc.vector.tensor_mul(
                            out=m_t[:, :n, :],
                            in0=tps[i][:, :n, :],
                            in1=xT[:, seg[0]:seg[0] + n, :],
                        )
                    else:
                        tcast = cbuf.tile([128, SEG, BS], BF16, tag="tc")
                        nc.scalar.copy(out=tcast[:, :n, :],
                                       in_=tps[i][:, :n, :])
                        if route == "pool":
                            # Pool multiplies from SBUF (it cannot read PSUM)
                            nc.gpsimd.tensor_mul(
                                out=m_t[:, :n, :],
                                in0=tcast[:, :n, :],
                                in1=xT[:, seg[0]:seg[0] + n, :],
                            )
                        else:
                            nc.vector.tensor_mul(
                                out=m_t[:, :n, :],
                                in0=tcast[:, :n, :],
                                in1=xT[:, seg[0]:seg[0] + n, :],
                            )
                    pending.append(
                        (m_t, [(u, CH_BY_IJG[(i, jg)]["k"])
                               for u, jg in enumerate(seg)]))
                    last_m = m_t
                for kind, arg in late_dmas.pop(sidx, []):
                    emit_late_dma(kind, arg, last_m)
            flush_dw2()

            # ---- tail: combine the 4 partial h's with one selector matmul
            # (ident4 doubles as the selector: hsum[d,b] = sum_g h4[32g+d,b])
            # h4's full 128 partitions are written (dw2 blocks are padded to
            # 32-wide strips), so one full-width copy drains it; the id4
            # selector zeroes the pad rows.
            hg_sb = lnp.tile([128, BS], F32, tag="hgsb")
            nc.scalar.copy(out=hg_sb[:], in_=hcomb[:, 0:BS])
            nc.tensor.matmul(hsum, lhsT=id4_t[:], rhs=hg_sb[:],
                             start=True, stop=True)
            hsum_sb = lnp.tile([D, BS], F32, tag="hsum_sb")
            nc.scalar.copy(out=hsum_sb[:], in_=hsum)

            def ht_v(half):
                # reuses h4's free range -- h4 is fully consumed by then
                off = half * D
                return hcomb[0:128, off:off + D]

            for half in range(2):
                nc.tensor.transpose(
                    ht_v(half),
                    hsum_sb[:, half * 128:(half + 1) * 128],
                    id4_t[0:D, :],
                )
            # ---- LayerNorm per 128-row half
            for half in range(2):
                hb = lnp.tile([128, D], F32, tag="hb")
                nc.vector.tensor_add(out=hb[:], in0=ht_v(half),
                                     in1=vec_t[:, 0, :])
                stats = lnp.tile([128, 6], F32, tag="stats")
                nc.vector.bn_stats(out=stats[:], in_=hb[:])
                mv = lnp.tile([128, 2], F32, tag="mv")
                nc.vector.bn_aggr(out=mv[:], in_=stats[:])
                nc.scalar.activation(
                    out=mv[:, 1:2], in_=mv[:, 1:2],
                    func=mybir.ActivationFunctionType.Sqrt,
                    bias=eps_t[:], scale=1.0,
                )
                nc.vector.reciprocal(out=mv[:, 1:2], in_=mv[:, 1:2])
                nc.vector.tensor_scalar(
                    out=hb[:], in0=hb[:],
                    scalar1=mv[:, 0:1], scalar2=mv[:, 1:2],
                    op0=mybir.AluOpType.subtract, op1=mybir.AluOpType.mult,
                )
                nc.vector.tensor_mul(out=hb[:], in0=hb[:], in1=vec_t[:, 1, :])
                nc.vector.tensor_add(out=hb[:], in0=hb[:], in1=vec_t[:, 2, :])
                nc.sync.dma_start(out=out[half * 128:(half + 1) * 128, :],
                                  in_=hb[:])
    nc.finalize()
    return nc


_NC_CACHE = None


def _get_nc():
    global _NC_CACHE
    if _NC_CACHE is None:
        _NC_CACHE = _build_bass()
    return _NC_CACHE


def run(x, W, dense_w, dense_b, gamma, beta, trace=False):
    x = np.asarray(x, np.float32)
    wsb_np, dw2_np = _host_weights(np.asarray(W, np.float32),
                                   np.asarray(dense_w, np.float32))
    vecs_np = np.stack([
        np.asarray(dense_b, np.float32),
        np.asarray(gamma, np.float32),
        np.asarray(beta, np.float32),
    ])
    id4_np = _host_ident4()
    in_maps = []
    for c in range(NCORES):
        in_maps.append({
            "xt": _host_xt(x[c * BS:(c + 1) * BS]),
            "wsb": wsb_np,
            "dw2": dw2_np,
            "vecs": vecs_np,
            "ident4": id4_np,
        })
    res = run_bass_kernel_spmd(
        _get_nc(), in_maps, core_ids=list(range(NCORES)), trace=trace
    )
    out = np.concatenate([res.results[c]["out"] for c in range(NCORES)], axis=0)
    return out.astype(np.float32), res


def kernel(x, W, dense_w, dense_b, gamma, beta):
    out, _ = run(x, W, dense_w, dense_b, gamma, beta)
    return out


# revision 48
# speedup vs baseline: 1.1781x; 1.1781x over previous
"""Trainium2 Bass kernel for BilinearInteractionPlusLayer.

Math (per batch row b):
    pairs (i,j), i<j over F=40 fields, P=C(40,2)=780 pairs
    t[b,p,f] = sum_e x[b,i,e] * W[p,e,f]
    q[b,p]   = sum_f t[b,p,f] * x[b,j,f]
    h[b,d]   = sum_p q[b,p] * dense_w[p,d] + dense_b[d]
    out      = LayerNorm(h) * gamma + beta          (eps = 1e-3)

Sharding: data-parallel over batch, 2048 -> 256 rows on each of 8 cores.
W / dense_w / LN params are replicated. No collectives.

Per-core pipeline (pair math in a transposed "[feature x batch]" layout):
  - Stage 1 (PE): per (field i, j-group jg) "chunk", one matmul
        lhsT = Wcat chunk [32e x 128(c,f)]  (host-packed, zeros for absent
               pairs, placed on partition strip i%4)
        rhs  = xT_i      [32e x 256b]
        out  = t chunk   [128(c,f) x 256b] in PSUM (fp32; TRN2 PSUM is
               fp32-only)
  - The elementwise q-multiply m = t * xj is routed per segment across
    three engine paths, statically balanced against measured HW rates:
        'act':  ACT casts t -> bf16 SBUF, DVE tensor_mul at the 2-byte
                2x fast path
        'pool': ACT casts, Pool (GpSimd) multiplies from SBUF
        'dve':  DVE multiplies fp32-from-PSUM directly (1x, no cast)
    Segments are 2 chunks (one PSUM bank) x 7 pool bufs, so a full
    4-strip step allocates without stalling the in-order PE queue.
  - PE folds dense_w AND the (pair,f) reduction into accumulating matmuls
    (4 column-strip partial accumulators run via tile_position packing).
  - Tail: transpose-accumulate into [b x d], LayerNorm via bn_stats/bn_aggr.
Startup: input DMAs are split across the two hardware DMA queues (Sync +
Scalar engines) in first-needed-first order, with the weight tensors split
into independently-DMA'd tiles so early matmuls only wait on their own
slice; the Sqrt activation table (which also contains Copy) is preloaded
with a dummy op so the LayerNorm tail doesn't pay a 1.3us table switch.
"""

import itertools

import numpy as np

import concourse.bass as bass
from concourse import bacc, mybir
from concourse.bass_utils import run_bass_kernel_spmd
from concourse.tile import TileContext

F32 = mybir.dt.float32
BF16 = mybir.dt.bfloat16
NP_BF16 = mybir.dt.np(BF16)

B, F, E, P, D = 2048, 40, 32, 780, 16
NCORES = 8
BS = B // NCORES          # 256 batch rows per core
NJG = F // 4              # 10 j-groups of 4 fields
LN_EPS = 1e-3
SEG = 2                   # j-group chunks per pipeline segment

# wsb tile boundaries (in slots) / dw2 tile boundaries (in chunks): each
# range is one SBUF tile fed by one DMA, so consumers only wait for the
# slice they read. All but the first are DMA'd from inside the main loop
# so early matmuls don't sit behind them on the queue's completion
# counter.
WSB_CUTS = [0, 12, 24, 36, 55]
DW2_CUTS = [0, 48, 112, 210]


def _make_chunks():
    """One chunk = (field i, j-group jg): a [32e x 128(c,f)] stage-1 matmul."""
    chunks = []
    per_strip = [0, 0, 0, 0]
    for i in range(F):
        r = i % 4
        for jg in range((i + 1) // 4, NJG):
            chunks.append(
                {"i": i, "jg": jg, "r": r, "s": per_strip[r], "k": len(chunks)}
            )
            per_strip[r] += 1
    return chunks, per_strip


CHUNKS, PER_STRIP = _make_chunks()
CH_BY_IJG = {(c["i"], c["jg"]): c for c in CHUNKS}
NCH = len(CHUNKS)          # 210
SLOTS = max(PER_STRIP)     # 55 weight-chunk slots per partition strip


def _segments_of(i):
    jgs = list(range((i + 1) // 4, NJG))
    return [jgs[s:s + SEG] for s in range(0, len(jgs), SEG)]


def _host_weights(W, dense_w):
    """Pack W into per-strip stationary chunks and dense_w into per-chunk
    [128(c,f) x 32] reducers (cols 0:16 = dense_w broadcast over f, cols
    16:32 = zeros so the dense matmuls write full 32-partition strips of
    h4, letting the tail drain h4 with one full-width copy)."""
    pair_idx = {pq: n for n, pq in enumerate(itertools.combinations(range(F), 2))}
    wsb = np.zeros((128, SLOTS * 128), np.float32)
    dw2 = np.zeros((128, NCH * 32), np.float32)
    for ch in CHUNKS:
        i, jg, r, s, k = ch["i"], ch["jg"], ch["r"], ch["s"], ch["k"]
        for c in range(4):
            j = 4 * jg + c
            if j <= i:
                continue
            p = pair_idx[(i, j)]
            wsb[32 * r:32 * r + 32, s * 128 + 32 * c:s * 128 + 32 * c + 32] = W[p]
            dw2[32 * c:32 * c + 32, k * 32:k * 32 + 16] = dense_w[p][None, :]
    return wsb.astype(NP_BF16), dw2.astype(NP_BF16)


def _host_xt(xc):
    """Per-core phase layout: xt[32*(j%4)+f, (j//4)*BS + b] = xc[b, j, f]."""
    arr = xc.transpose(1, 2, 0)                    # [F, E, BS]
    arr = arr.reshape(NJG, 4, E, BS)               # [jg, c, f, b]
    arr = arr.transpose(1, 2, 0, 3)                # [c, f, jg, b]
    return np.ascontiguousarray(arr.reshape(128, NJG * BS)).astype(NP_BF16)


def _host_ident4():
    """[128, 16] with a 16x16 identity at each 32-partition strip."""
    id4 = np.zeros((128, 16), np.float32)
    for g in range(4):
        id4[32 * g:32 * g + 16, :] = np.eye(16, dtype=np.float32)
    return id4


def _build_steps():
    """Baseline step schedule: quads of consecutive fields, software
    pipelined; returns the list of steps, each a list of (i, seg-jgs)."""
    steps = []
    for t0 in range(0, F, 4):
        quad = [i for i in range(t0, min(t0 + 4, F))]
        segl = {i: _segments_of(i) for i in quad}
        nsteps = max(len(s) for s in segl.values())
        for step in range(nsteps):
            steps.append([(i, segl[i][step]) for i in quad
                          if step < len(segl[i])])
    return steps


def _route_schedule(steps):
    """Statically balance the elementwise work across ACT / DVE / POOL.
    Returns {group_index: route} where groups enumerate (i, seg) in
    emission order. Routes: 'act' (ACT cast + DVE fast mul), 'pool'
    (Pool cast + DVE fast mul), 'dve' (DVE direct fp32 mul)."""
    # per-engine busy seeds (ns): tail work living on each engine
    busy = {"ACT": 2600.0, "DVE": 2400.0, "POOL": 400.0}
    routes = []
    for active in steps:
        for i, seg in active:
            n = len(seg)
            # measured per-chunk rates: ACT cast 213, DVE bf16 mul 133 (2x),
            # DVE direct fp32 mul 267 (1x), Pool bf16 mul 508
            cand = {
                "act": {"ACT": 213 * n + 260, "DVE": 133 * n + 125},
                "pool": {"ACT": 213 * n + 260, "POOL": 508 * n + 270},
                "dve": {"DVE": 267 * n + 150},
            }
            best, best_max = None, None
            for r, costs in cand.items():
                trial = dict(busy)
                for eng, c in costs.items():
                    trial[eng] += c
                m = max(trial.values())
                if best_max is None or m < best_max:
                    best, best_max = r, m
            for eng, c in cand[best].items():
                busy[eng] += c
            routes.append(best)
    return routes, busy


def _build_bass():
    nc = bacc.Bacc(trn_type="TRN2")
    xin = nc.dram_tensor("xt", [128, NJG * BS], BF16, kind="ExternalInput")
    wsb = nc.dram_tensor("wsb", [128, SLOTS * 128], BF16, kind="ExternalInput")
    dw2 = nc.dram_tensor("dw2", [128, NCH * 32], BF16, kind="ExternalInput")
    vecs = nc.dram_tensor("vecs", [3, D], F32, kind="ExternalInput")
    id4 = nc.dram_tensor("ident4", [128, D], F32, kind="ExternalInput")
    out = nc.dram_tensor("out", [BS, D], F32, kind="ExternalOutput")

    steps = _build_steps()
    routes, pred_busy = _route_schedule(steps)
    import collections
    import sys
    print(f"route mix: {dict(collections.Counter(routes))}  "
          f"predicted busy(ns): { {k: int(v) for k, v in pred_busy.items()} }",
          file=sys.stderr)

    with TileContext(nc) as tc:
        with (
            tc.tile_pool(name="const", bufs=1) as const,
            tc.tile_pool(name="cast", bufs=12) as cbuf,
            tc.tile_pool(name="mbuf", bufs=18) as mbuf,
            tc.tile_pool(name="tsegp", bufs=7, space="PSUM") as tsegp,
            tc.tile_pool(name="hpsp", bufs=1, space="PSUM") as hpsp,
            tc.tile_pool(name="lnp", bufs=2) as lnp,
        ):
            # ---- inputs. The Sync hardware queue carries ONLY what the
            # first matmuls need (the tile scheduler hoists every queued
            # DMA ahead of compute, and consumers wait on the queue's
            # completion count, so anything extra here delays the start).
            # Everything else rides the GpSimd software DGE in
            # consumption order. The Scalar engine gets no DMAs (a DMA
            # occupies the issuing engine for the whole transfer, which
            # would starve the casts).
            xT = const.tile([128, NJG, BS], BF16)
            wsb_tiles = [
                (a, b_, const.tile([128, (b_ - a) * 128], BF16, name=f"wsb{a}"))
                for a, b_ in zip(WSB_CUTS[:-1], WSB_CUTS[1:])
            ]
            dw2_tiles = [
                (a, b_, const.tile([128, (b_ - a) * 32], BF16, name=f"dw2{a}"))
                for a, b_ in zip(DW2_CUTS[:-1], DW2_CUTS[1:])
            ]
            nc.sync.dma_start(out=xT[:, 0:2, :], in_=xin[:, 0:2 * BS])
            a, b_, t_ = wsb_tiles[0]
            nc.sync.dma_start(out=t_[:], in_=wsb[:, a * 128:b_ * 128])
            nc.sync.dma_start(out=xT[:, 2:NJG, :], in_=xin[:, 2 * BS:])
            vec_t = const.tile([128, 3, D], F32)
            id4_t = const.tile([128, D], F32)
            # Late DMAs are emitted from inside the loop; each is pinned
            # behind real compute by a 1-element guard copy into its
            # destination (a WAW dep the scheduler cannot hoist past), so
            # early matmuls never wait on the shared DMA counter for them.
            def emit_late_dma(kind, arg, last_m):
                if kind == "wsb":
                    a, b_, t_ = arg
                    nc.gpsimd.tensor_copy(out=t_[0:1, 0:1],
                                          in_=last_m[0:1, 0, 0:1])
                    nc.sync.dma_start(out=t_[:], in_=wsb[:, a * 128:b_ * 128])
                elif kind == "dw2":
                    a, b_, t_ = arg
                    nc.gpsimd.tensor_copy(out=t_[0:1, 0:1],
                                          in_=last_m[0:1, 0, 0:1])
                    nc.sync.dma_start(out=t_[:], in_=dw2[:, a * 32:b_ * 32])
                elif kind == "vecs":
                    # rows: 0 = dense_b, 1 = gamma, 2 = beta; broadcast
                    # across partitions
                    nc.gpsimd.tensor_copy(out=vec_t[0:1, 0, 0:1],
                                          in_=last_m[0:1, 0, 0:1])
                    src = vecs[:, :]
                    nc.sync.dma_start(
                        out=vec_t[:],
                        in_=bass.AP(tensor=src.tensor, offset=src.offset,
                                    ap=[[0, 128]] + [list(a) for a in src.ap]),
                    )
                else:
                    nc.gpsimd.tensor_copy(out=id4_t[0:1, 0:1],
                                          in_=last_m[0:1, 0, 0:1])
                    nc.sync.dma_start(out=id4_t[:], in_=id4[:, :])

            late_dmas = {
                1: [("dw2", dw2_tiles[0]), ("wsb", wsb_tiles[1])],
                3: [("dw2", dw2_tiles[1]), ("wsb", wsb_tiles[2])],
                5: [("dw2", dw2_tiles[2]), ("wsb", wsb_tiles[3])],
                7: [("vecs", None)],
                9: [("id4", None)],
            }
            eps_t = const.tile([128, 1], F32)
            nc.vector.memset(eps_t[:], LN_EPS)

            def wsb_ap(s):
                for a, b_, t_ in wsb_tiles:
                    if a <= s < b_:
                        return t_, (s - a)
                raise AssertionError(s)

            def dw2_ap(k):
                for a, b_, t_ in dw2_tiles:
                    if a <= k < b_:
                        return t_, (k - a)
                raise AssertionError(k)

            def rr_order(items):
                """Round-robin dw2 items across the 4 column strips so
                adjacent matmuls hit different PE column tiles."""
                buckets = {g: [] for g in range(4)}
                for it in items:
                    buckets[it[2] % 4].append(it)
                out_ = []
                while any(buckets.values()):
                    for g in range(4):
                        if buckets[g]:
                            out_.append(buckets[g].pop(0))
                return out_

            # One PSUM bank holds: h4 (four col-tiled partial accumulators
            # [16d x 256b] at partition strips, free 0:256), hsum ([16d x
            # 256b], free 256:512), and ht reuses h4's range after h4 is
            # consumed.
            hcomb = hpsp.tile([128, 2 * BS], F32)
            h4 = hcomb[:, 0:BS]
            hsum = hcomb[0:D, BS:2 * BS]

            # Prepass: replay the flush schedule to learn the actual
            # per-strip first/last dw2 matmul (for PSUM start/stop flags).
            sim_pending = []
            sim_order = []

            def sim_flush():
                items = [(None, u, CH_BY_IJG[(i, jg)]["k"])
                         for (i, seg) in sim_pending for u, jg in enumerate(seg)]
                sim_order.extend(k for _, _, k in rr_order(items))
                sim_pending.clear()

            for sidx, active in enumerate(steps):
                if sidx % 2 == 0 and sidx > 0:
                    sim_flush()
                sim_pending.extend(active)
            sim_flush()
            first_k = {}
            last_k = {}
            for k in sim_order:
                first_k.setdefault(k % 4, k)
                last_k[k % 4] = k

            pending = []          # list of (m_tile, [(u, k), ...])

            def flush_dw2():
                items = [(m_t, u, k) for m_t, ks in pending for u, k in ks]
                for m_t, u, k in rr_order(items):
                    g = k % 4
                    dtile, koff = dw2_ap(k)
                    nc.tensor.matmul(
                        h4[32 * g:32 * g + 32, :],
                        lhsT=dtile[:, koff * 32:(koff + 1) * 32],
                        rhs=m_t[:, u, :],
                        start=(first_k[g] == k),
                        stop=(last_k[g] == k),
                        tile_position=(0, 32 * g),
                    )
                pending.clear()

            gidx = 0
            for sidx, active in enumerate(steps):
                tps = {i: tsegp.tile([128, SEG, BS], F32, tag="t",
                                     name="tseg")
                       for i, _ in active}
                # stage-1 matmul burst, chunk-interleaved across strips
                maxc = max(len(seg) for _, seg in active)
                for u in range(maxc):
                    for i, seg in active:
                        if u >= len(seg):
                            continue
                        ch = CH_BY_IJG[(i, seg[u])]
                        r = i % 4
                        wtile, soff = wsb_ap(ch["s"])
                        nc.tensor.matmul(
                            tps[i][:, u, :],
                            lhsT=wtile[32 * r:32 * r + 32,
                                       soff * 128:(soff + 1) * 128],
                            rhs=xT[32 * r:32 * r + 32, i // 4, :],
                            start=True, stop=True,
                            tile_position=(32 * r, 0),
                        )
                # dw2 matmuls are flushed every other step (with >=1 step of
                # lag so the m tiles are ready): longer same-kind PE runs
                # mean fewer row-tile <-> column-tile transitions, which
                # serialize the PE array.
                if sidx % 2 == 0 and sidx > 0:
                    flush_dw2()
                # elementwise multiply, statically routed
                for i, seg in active:
                    n = len(seg)
                    route = routes[gidx]
                    gidx += 1
                    m_t = mbuf.tile([128, SEG, BS], BF16, tag="m")
                    if route == "dve":
                        nc.vector.tensor_mul(
                            out=m_t[:, :n, :],
                            in0=tps[i][:, :n, :],
                            in1=xT[:, seg[0]:seg[0] + n, :],
                        )
                    else:
                        tcast = cbuf.tile([128, SEG, BS], BF16, tag="tc")
                        nc.scalar.copy(out=tcast[:, :n, :],
                                       in_=tps[i][:, :n, :])
                        if route == "pool":
                            # Pool multiplies from SBUF (it cannot read PSUM)
                            nc.gpsimd.tensor_mul(
                                out=m_t[:, :n, :],
                                in0=tcast[:, :n, :],
                                in1=xT[:, seg[0]:seg[0] + n, :],
                            )
                        else:
                            nc.vector.tensor_mul(
                                out=m_t[:, :n, :],
                                in0=tcast[:, :n, :],
                                in1=xT[:, seg[0]:seg[0] + n, :],
                            )
                    pending.append(
                        (m_t, [(u, CH_BY_IJG[(i, jg)]["k"])
                               for u, jg in enumerate(seg)]))
                    last_m = m_t
                for kind, arg in late_dmas.pop(sidx, []):
                    emit_late_dma(kind, arg, last_m)
            flush_dw2()

            # ---- tail: combine the 4 partial h's with one selector matmul
            # (ident4 doubles as the selector: hsum[d,b] = sum_g h4[32g+d,b])
            # h4's full 128 partitions are written (dw2 blocks are padded to
            # 32-wide strips), so one full-width copy drains it; the id4
            # selector zeroes the pad rows.
            hg_sb = lnp.tile([128, BS], F32, tag="hgsb")
            nc.scalar.copy(out=hg_sb[:], in_=hcomb[:, 0:BS])
            nc.tensor.matmul(hsum, lhsT=id4_t[:], rhs=hg_sb[:],
                             start=True, stop=True)
            hsum_sb = lnp.tile([D, BS], F32, tag="hsum_sb")
            nc.scalar.copy(out=hsum_sb[:], in_=hsum)

            def ht_v(half):
                # reuses h4's free range -- h4 is fully consumed by then
                off = half * D
                return hcomb[0:128, off:off + D]

            for half in range(2):
                nc.tensor.transpose(
                    ht_v(half),
                    hsum_sb[:, half * 128:(half + 1) * 128],
                    id4_t[0:D, :],
                )
            # ---- LayerNorm per 128-row half
            for half in range(2):
                hb = lnp.tile([128, D], F32, tag="hb")
                nc.vector.tensor_add(out=hb[:], in0=ht_v(half),
                                     in1=vec_t[:, 0, :])
                stats = lnp.tile([128, 6], F32, tag="stats")
                nc.vector.bn_stats(out=stats[:], in_=hb[:])
                mv = lnp.tile([128, 2], F32, tag="mv")
                nc.vector.bn_aggr(out=mv[:], in_=stats[:])
                nc.scalar.activation(
                    out=mv[:, 1:2], in_=mv[:, 1:2],
                    func=mybir.ActivationFunctionType.Sqrt,
                    bias=eps_t[:], scale=1.0,
                )
                nc.vector.reciprocal(out=mv[:, 1:2], in_=mv[:, 1:2])
                nc.vector.tensor_scalar(
                    out=hb[:], in0=hb[:],
                    scalar1=mv[:, 0:1], scalar2=mv[:, 1:2],
                    op0=mybir.AluOpType.subtract, op1=mybir.AluOpType.mult,
                )
                nc.vector.tensor_mul(out=hb[:], in0=hb[:], in1=vec_t[:, 1, :])
                nc.vector.tensor_add(out=hb[:], in0=hb[:], in1=vec_t[:, 2, :])
                nc.sync.dma_start(out=out[half * 128:(half + 1) * 128, :],
                                  in_=hb[:])
    nc.finalize()
    return nc


_NC_CACHE = None


def _get_nc():
    global _NC_CACHE
    if _NC_CACHE is None:
        _NC_CACHE = _build_bass()
    return _NC_CACHE


def run(x, W, dense_w, dense_b, gamma, beta, trace=False):
    x = np.asarray(x, np.float32)
    wsb_np, dw2_np = _host_weights(np.asarray(W, np.float32),
                                   np.asarray(dense_w, np.float32))
    vecs_np = np.stack([
        np.asarray(dense_b, np.float32),
        np.asarray(gamma, np.float32),
        np.asarray(beta, np.float32),
    ])
    id4_np = _host_ident4()
    in_maps = []
    for c in range(NCORES):
        in_maps.append({
            "xt": _host_xt(x[c * BS:(c + 1) * BS]),
            "wsb": wsb_np,
            "dw2": dw2_np,
            "vecs": vecs_np,
            "ident4": id4_np,
        })
    res = run_bass_kernel_spmd(
        _get_nc(), in_maps, core_ids=list(range(NCORES)), trace=trace
    )
    out = np.concatenate([res.results[c]["out"] for c in range(NCORES)], axis=0)
    return out.astype(np.float32), res


def kernel(x, W, dense_w, dense_b, gamma, beta):
    out, _ = run(x, W, dense_w, dense_b, gamma, beta)
    return out
